# revision 3
# baseline (speedup 1.0000x reference)
"""Trainium2 Bass kernel for nn_MultiHeadAttention (B=4, S=2048, D=1024, H=16).

Sharding: 8 cores = batch (4) x query-row-halves (2). Each core computes, for
its batch b and its 1024 query rows: the QKV projections (K/V over the full
2048 key rows), all 16 heads of attention, the output projection, residual add
and LayerNorm. No collectives needed - every core's output rows depend only on
its own data. The full attention-probability tensor [4,16,2048,2048] is a
kernel output; each core writes its [16,1024,2048] slice.

Per-core dataflow (all matmul inputs bf16, fp32 PSUM accumulation):
  P0: DMA-transpose-load inputs (d on partitions), project to
      QT/KT [head_dim-major, seq] and V [seq-major, head_dim] with a ones
      column appended per head (yields softmax denominators for free).
  P1: per (head, q-half) generation: scores^T = K Q^T / 8 on PE, exp on ACT,
      attn@V accumulation on PE (ones row -> denominators), PE-transpose of
      the exp tiles back to [q, k] orientation, normalize on DVE, DMA out.
      Context rows are normalized per head (pre head-mix) via a DRAM-scratch
      broadcast of 1/sum.
  P2: out-projection on PE, + bias + residual, LayerNorm (bn_stats/bn_aggr),
      DMA out rows.
"""

import sys

sys.path.insert(0, "/opt/trn_rl_repo")

import numpy as np
import ml_dtypes

import concourse.bass as bass
import concourse.mybir as mybir
import concourse.tile as tile
from concourse.masks import make_identity
from concourse.bass_utils import run_bass_kernel_spmd

F32 = mybir.dt.float32
BF16 = mybir.dt.bfloat16
AF = mybir.ActivationFunctionType
ALU = mybir.AluOpType

B, S, D, H = 4, 2048, 1024, 16
HD = D // H          # 64
SQ = S // 2          # local query rows per core = 1024
N_CORES = 8


def bcast_ap(dram_param, parts, free):
    """DRAM [free] -> AP replicated across `parts` partitions."""
    return bass.AP(tensor=dram_param[:].tensor, offset=0, ap=[[0, parts], [1, free]])


def build_nc():
    nc = bass.Bass()

    qpb = nc.declare_dram_parameter("qpb", [SQ, D], BF16, isOutput=False)
    kpb = nc.declare_dram_parameter("kpb", [S, D], BF16, isOutput=False)
    vpb = nc.declare_dram_parameter("vpb", [S, D], BF16, isOutput=False)
    qres = nc.declare_dram_parameter("qres", [SQ, D], F32, isOutput=False)
    wqT = nc.declare_dram_parameter("wqT", [D, D], BF16, isOutput=False)
    wkT = nc.declare_dram_parameter("wkT", [D, D], BF16, isOutput=False)
    wvT = nc.declare_dram_parameter("wvT", [D, D], BF16, isOutput=False)
    woT = nc.declare_dram_parameter("woT", [D, D], BF16, isOutput=False)
    bq_d = nc.declare_dram_parameter("bq", [D], F32, isOutput=False)
    bk_d = nc.declare_dram_parameter("bk", [D], F32, isOutput=False)
    bv_d = nc.declare_dram_parameter("bv", [D], F32, isOutput=False)
    bo_d = nc.declare_dram_parameter("bo", [D], F32, isOutput=False)
    gamma_d = nc.declare_dram_parameter("gamma", [D], F32, isOutput=False)
    beta_d = nc.declare_dram_parameter("beta", [D], F32, isOutput=False)

    attn_out = nc.declare_dram_parameter("attn_out", [H, SQ, S], F32, isOutput=True)
    out_rows = nc.declare_dram_parameter("out_rows", [SQ, D], F32, isOutput=True)

    with tile.TileContext(nc) as tc:
        with tc.tile_pool(name="res", bufs=1) as res, \
             tc.tile_pool(name="ps_mm", bufs=2, space="PSUM") as ps_mm, \
             tc.tile_pool(name="ps_ctx", bufs=2, space="PSUM") as ps_ctx, \
             tc.tile_pool(name="ps_tr", bufs=2, space="PSUM") as ps_tr:
            # ---------------- residents ----------------
            ident = res.tile([128, 128], BF16)
            make_identity(nc, ident)

            QT = res.tile([128, 8, SQ], BF16)      # [hd-part, i-chunk, q]
            KT = res.tile([128, 8, S], BF16)       # [hd-part, i-chunk, k]
            Vaug = res.tile([128, 16, H, HD + 1], BF16)  # [k-part, ktile, head, hd+1]
            ctxT = res.tile([128, 8, SQ], BF16)    # like QT
            woT_sb = res.tile([128, 8, D], BF16)   # [i-part, i-chunk, dout]

            bq_sb = res.tile([128, 8], F32)
            bk_sb = res.tile([128, 8], F32)
            bv_b = res.tile([128, D], F32)
            bo_b = res.tile([128, D], F32)
            g_b = res.tile([128, D], F32)
            be_b = res.tile([128, D], F32)
            eps = res.tile([128, 1], F32)

            nc.vector.memset(eps, 1e-5)
            # ones column for the denominators: fill all of Vaug with 1.0,
            # the projection copies below overwrite cols 0..63 per head.
            nc.vector.memset(Vaug, 1.0)

            for dst, src in ((bq_sb, bq_d), (bk_sb, bk_d)):
                nc.sync.dma_start(
                    out=dst,
                    in_=bass.AP(tensor=src[:].tensor, offset=0, ap=[[1, 128], [128, 8]]),
                )
            for dst, src in ((bv_b, bv_d), (bo_b, bo_d), (g_b, gamma_d), (be_b, beta_d)):
                nc.sync.dma_start(out=dst, in_=bcast_ap(src, 128, D))

            nc.sync.dma_start(
                out=woT_sb, in_=woT[:].rearrange("(c p) o -> p c o", p=128)
            )

            # ---------------- P0: transpose-loads + projections ----------------
            with tc.tile_pool(name="p0", bufs=1) as p0, \
                 tc.tile_pool(name="p0c", bufs=2) as p0c, \
                 tc.tile_pool(name="p0w", bufs=1) as p0w:
                # Q and K projections -> transposed layouts
                for xpb, s_x, wT_d, dest, bias_sb in (
                    (qpb, SQ, wqT, QT, bq_sb),
                    (kpb, S, wkT, KT, bk_sb),
                ):
                    xT = p0.tile([128, 8, S], BF16, tag="xT")
                    for dc in range(8):
                        nc.sync.dma_start(
                            out=xT[:, dc, 0:s_x],
                            in_=xpb[:, dc * 128:(dc + 1) * 128],
                            transpose=True,
                        )
                    for ic in range(8):
                        w_col = p0c.tile([128, 8, 128], BF16, tag="wcol")
                        nc.sync.dma_start(
                            out=w_col,
                            in_=wT_d[:, ic * 128:(ic + 1) * 128].rearrange(
                                "(c p) i -> p c i", p=128
                            ),
                        )
                        for sc in range(s_x // 512):
                            ps = ps_mm.tile([128, 512], F32, tag="mm")
                            for dc in range(8):
                                nc.tensor.matmul(
                                    ps,
                                    w_col[:, dc, :],
                                    xT[:, dc, sc * 512:(sc + 1) * 512],
                                    start=(dc == 0),
                                    stop=(dc == 7),
                                )
                            nc.scalar.activation(
                                out=dest[:, ic, sc * 512:(sc + 1) * 512],
                                in_=ps,
                                func=AF.Identity,
                                bias=bias_sb[:, ic:ic + 1],
                            )

                # V projection -> natural layout with ones column
                xT = p0.tile([128, 8, S], BF16, tag="xT")
                for dc in range(8):
                    nc.sync.dma_start(
                        out=xT[:, dc, :],
                        in_=vpb[:, dc * 128:(dc + 1) * 128],
                        transpose=True,
                    )
                wv_sb = p0w.tile([128, 8, D], BF16)
                nc.sync.dma_start(
                    out=wv_sb, in_=wvT[:].rearrange("(c p) i -> p c i", p=128)
                )
                for st in range(16):
                    for ic2 in range(2):
                        ps = ps_mm.tile([128, 512], F32, tag="mm")
                        for dc in range(8):
                            nc.tensor.matmul(
                                ps,
                                xT[:, dc, st * 128:(st + 1) * 128],
                                wv_sb[:, dc, ic2 * 512:(ic2 + 1) * 512],
                                start=(dc == 0),
                                stop=(dc == 7),
                            )
                        nc.vector.tensor_add(
                            out=Vaug[:, st, ic2 * 8:(ic2 + 1) * 8, 0:HD],
                            in0=ps.rearrange("p (h e) -> p h e", h=8),
                            in1=bv_b[:, ic2 * 512:(ic2 + 1) * 512].rearrange(
                                "p (h e) -> p h e", h=8
                            ),
                        )

            # ---------------- P1: attention, per (head, q-half) ----------------
            with tc.tile_pool(name="p1e", bufs=2) as p1e, \
                 tc.tile_pool(name="p1s", bufs=3) as p1s, \
                 tc.tile_pool(name="p1m", bufs=3) as p1m, \
                 tc.tile_pool(name="p1d", bufs=3, space="DRAM") as p1d:
                for h in range(H):
                    hp = (h % 2) * 64
                    hc = h // 2
                    for qh in range(2):
                        qoff = qh * 512
                        eT = p1e.tile([128, 16, 512], BF16, tag="expT")
                        pc = ps_ctx.tile([HD + 1, 512], F32, tag="ctx")
                        for ktp in range(8):
                            ps = ps_mm.tile([128, 1024], F32, tag="mm")
                            for k2 in range(2):
                                kt = ktp * 2 + k2
                                nc.tensor.matmul(
                                    ps[:, k2 * 512:(k2 + 1) * 512],
                                    KT[hp:hp + 64, hc, kt * 128:(kt + 1) * 128],
                                    QT[hp:hp + 64, hc, qoff:qoff + 512],
                                    start=True,
                                    stop=True,
                                )
                            nc.scalar.activation(
                                out=eT[:, ktp * 2:ktp * 2 + 2, :].rearrange(
                                    "p a b -> p (a b)"
                                ),
                                in_=ps,
                                func=AF.Exp,
                                scale=0.125,
                            )
                            for k2 in range(2):
                                kt = ktp * 2 + k2
                                nc.tensor.matmul(
                                    pc,
                                    Vaug[:, kt, h, :],
                                    eT[:, kt, :],
                                    start=(kt == 0),
                                    stop=(kt == 15),
                                )
                        # denominators -> 1/sum, in two layouts
                        rrow = p1m.tile([1, 512], F32, tag="row")
                        nc.vector.reciprocal(out=rrow, in_=pc[HD:HD + 1, :])
                        scr = p1d.tile([1, 512], F32, tag="scr")
                        nc.sync.dma_start(out=scr, in_=rrow)
                        rsT = p1m.tile([128, 4], F32, tag="rsT")
                        nc.sync.dma_start(
                            out=rsT,
                            in_=bass.AP(
                                tensor=scr.tensor, offset=scr.offset,
                                ap=[[1, 128], [128, 4]],
                            ),
                        )
                        rb = p1m.tile([64, 512], F32, tag="rb")
                        nc.sync.dma_start(
                            out=rb,
                            in_=bass.AP(
                                tensor=scr.tensor, offset=scr.offset,
                                ap=[[0, 64], [1, 512]],
                            ),
                        )
                        # normalized context rows for this head
                        nc.vector.tensor_mul(
                            out=ctxT[hp:hp + 64, hc, qoff:qoff + 512],
                            in0=pc[0:HD, :],
                            in1=rb,
                        )
                        # attn tiles: transpose exp back to [q, k], scale, store
                        for qt in range(4):
                            stg = p1s.tile([128, S], F32, tag="stage")
                            for grp in range(2):
                                ptr = ps_tr.tile([128, 1024], BF16, tag="tr")
                                for j in range(8):
                                    kt = grp * 8 + j
                                    nc.tensor.transpose(
                                        ptr[:, j * 128:(j + 1) * 128],
                                        eT[:, kt, qt * 128:(qt + 1) * 128],
                                        ident,
                                    )
                                nc.vector.tensor_scalar_mul(
                                    out=stg[:, grp * 1024:(grp + 1) * 1024],
                                    in0=ptr,
                                    scalar1=rsT[:, qt:qt + 1],
                                )
                            nc.sync.dma_start(
                                out=attn_out[h, qoff + qt * 128:qoff + (qt + 1) * 128, :],
                                in_=stg,
                            )

            # ---------------- P2: out-projection + residual + LayerNorm --------
            with tc.tile_pool(name="p2", bufs=2) as p2:
                for st in range(8):
                    po = ps_mm.tile([128, 1024], F32, tag="mm")
                    for oc in range(2):
                        for ic in range(8):
                            nc.tensor.matmul(
                                po[:, oc * 512:(oc + 1) * 512],
                                ctxT[:, ic, st * 128:(st + 1) * 128],
                                woT_sb[:, ic, oc * 512:(oc + 1) * 512],
                                start=(ic == 0),
                                stop=(ic == 7),
                            )
                    qrt = p2.tile([128, D], F32, tag="qres")
                    nc.sync.dma_start(
                        out=qrt, in_=qres[st * 128:(st + 1) * 128, :]
                    )
                    xt2 = p2.tile([128, D], F32, tag="xt")
                    nc.vector.tensor_add(out=xt2, in0=po, in1=bo_b)
                    nc.vector.tensor_add(out=xt2, in0=xt2, in1=qrt)
                    xv = xt2.rearrange("p (s f) -> p s f", s=2)
                    stats = p2.tile([128, 2, 6], F32, tag="stats")
                    for s_ in range(2):
                        nc.vector.bn_stats(out=stats[:, s_, :], in_=xv[:, s_, :])
                    mv = p2.tile([128, 2], F32, tag="mv")
                    nc.vector.bn_aggr(out=mv, in_=stats)
                    std = p2.tile([128, 1], F32, tag="std")
                    nc.scalar.activation(
                        out=std, in_=mv[:, 1:2], func=AF.Sqrt, bias=eps
                    )
                    rstd = p2.tile([128, 1], F32, tag="rstd")
                    nc.vector.reciprocal(out=rstd, in_=std)
                    nc.vector.tensor_scalar(
                        out=xt2, in0=xt2, scalar1=mv[:, 0:1], scalar2=rstd,
                        op0=ALU.subtract, op1=ALU.mult,
                    )
                    nc.vector.tensor_mul(out=xt2, in0=xt2, in1=g_b)
                    nc.vector.tensor_add(out=xt2, in0=xt2, in1=be_b)
                    nc.sync.dma_start(
                        out=out_rows[st * 128:(st + 1) * 128, :], in_=xt2
                    )

    split_excess_waits(nc)
    return nc


def split_excess_waits(nc, cap=1):
    """This walrus build rejects >cap sem waits on one instruction ("Too many
    sync wait commands"). Move excess waits onto preceding NoOps on the same
    engine - engine program order makes this equivalent."""
    n_split = 0
    for fn in nc.m.functions:
        for blk in fn.blocks:
            if not any(
                inst.sync_info is not None and len(inst.sync_info.on_wait) > cap
                for inst in blk.instructions
            ):
                continue
            newins = []
            for inst in blk.instructions:
                si = inst.sync_info
                waits = list(si.on_wait) if si is not None else []
                if len(waits) > cap:
                    extra, keep = waits[:-cap], waits[-cap:]
                    for w in extra:
                        nop = mybir.InstNoOp(
                            name=f"{inst.name}-ws{n_split}", ins=[], outs=[]
                        )
                        nop.engine = inst.engine
                        nop.sync_info = mybir.SyncInfo(on_wait=[w], on_update=[])
                        newins.append(nop)
                        n_split += 1
                    inst.sync_info = mybir.SyncInfo(
                        on_wait=keep, on_update=list(si.on_update)
                    )
                newins.append(inst)
            blk.instructions = newins
    return n_split


_NC_CACHE = None


def _get_nc():
    global _NC_CACHE
    if _NC_CACHE is None:
        _NC_CACHE = build_nc()
    return _NC_CACHE


def make_in_maps(query, key_t, value, Wq, bq, Wk, bk, Wv, bv, Wo, bo, gamma, beta):
    bf16 = ml_dtypes.bfloat16
    query = np.asarray(query, dtype=np.float32)
    key_t = np.asarray(key_t, dtype=np.float32)
    value = np.asarray(value, dtype=np.float32)
    shared = {
        "wqT": np.ascontiguousarray(np.asarray(Wq, np.float32).T).astype(bf16),
        "wkT": np.ascontiguousarray(np.asarray(Wk, np.float32).T).astype(bf16),
        "wvT": np.ascontiguousarray(np.asarray(Wv, np.float32).T).astype(bf16),
        "woT": np.ascontiguousarray(np.asarray(Wo, np.float32).T).astype(bf16),
        "bq": np.asarray(bq, np.float32),
        "bk": np.asarray(bk, np.float32),
        "bv": np.asarray(bv, np.float32),
        "bo": np.asarray(bo, np.float32),
        "gamma": np.asarray(gamma, np.float32),
        "beta": np.asarray(beta, np.float32),
    }
    in_maps = []
    for c in range(N_CORES):
        b, hf = c // 2, c % 2
        rows = slice(hf * SQ, (hf + 1) * SQ)
        in_maps.append(
            {
                "qpb": query[b, rows].astype(bf16),
                "kpb": key_t[b].astype(bf16),
                "vpb": value[b].astype(bf16),
                "qres": np.ascontiguousarray(query[b, rows]),
                **shared,
            }
        )
    return in_maps


def run(in_maps, **kw):
    nc = _get_nc()
    return run_bass_kernel_spmd(nc, in_maps, list(range(N_CORES)), **kw)


def assemble(results):
    out = np.empty((B, S, D), np.float32)
    attn = np.empty((B, H, S, S), np.float32)
    for c in range(N_CORES):
        b, hf = c // 2, c % 2
        rows = slice(hf * SQ, (hf + 1) * SQ)
        out[b, rows, :] = results[c]["out_rows"]
        attn[b, :, rows, :] = results[c]["attn_out"]
    return out, attn


def kernel(query, key_t, value, Wq, bq, Wk, bk, Wv, bv, Wo, bo, gamma, beta):
    in_maps = make_in_maps(
        query, key_t, value, Wq, bq, Wk, bk, Wv, bv, Wo, bo, gamma, beta
    )
    res = run(in_maps)
    return assemble(res.results)


# revision 4
# speedup vs baseline: 1.0439x; 1.0439x over previous
"""Trainium2 Bass kernel for nn_MultiHeadAttention (B=4, S=2048, D=1024, H=16).

Sharding: 8 cores = batch (4) x query-row-halves (2). Each core computes, for
its batch b and its 1024 query rows: the QKV projections (K/V over the full
2048 key rows), all 16 heads of attention, the output projection, residual add
and LayerNorm. No collectives needed - every core's output rows depend only on
its own data. The full attention-probability tensor [4,16,2048,2048] is a
kernel output; each core writes its [16,1024,2048] slice.

Per-core dataflow (all matmul inputs bf16, fp32 PSUM accumulation):
  P0: DMA-transpose-load inputs (d on partitions), project to
      QT/KT [head_dim-major, seq] and V [seq-major, head_dim] with a ones
      column appended per head (yields softmax denominators for free).
  P1: per (head, q-half) generation: scores^T = K Q^T / 8 on PE, exp on ACT,
      attn@V accumulation on PE (ones row -> denominators), PE-transpose of
      the exp tiles back to [q, k] orientation, normalize on DVE, DMA out.
      Context rows are normalized per head (pre head-mix) via a DRAM-scratch
      broadcast of 1/sum.
  P2: out-projection on PE, + bias + residual, LayerNorm (bn_stats/bn_aggr),
      DMA out rows.
"""

import sys

sys.path.insert(0, "/opt/trn_rl_repo")

import numpy as np
import ml_dtypes

import concourse.bass as bass
import concourse.mybir as mybir
import concourse.tile as tile
from concourse.masks import make_identity
from concourse.bass_utils import run_bass_kernel_spmd

F32 = mybir.dt.float32
BF16 = mybir.dt.bfloat16
AF = mybir.ActivationFunctionType
ALU = mybir.AluOpType

B, S, D, H = 4, 2048, 1024, 16
HD = D // H          # 64
SQ = S // 2          # local query rows per core = 1024
N_CORES = 8


def bcast_ap(dram_param, parts, free):
    """DRAM [free] -> AP replicated across `parts` partitions."""
    return bass.AP(tensor=dram_param[:].tensor, offset=0, ap=[[0, parts], [1, free]])


def build_nc():
    nc = bass.Bass()

    qpb = nc.declare_dram_parameter("qpb", [SQ, D], BF16, isOutput=False)
    kpb = nc.declare_dram_parameter("kpb", [S, D], BF16, isOutput=False)
    vpb = nc.declare_dram_parameter("vpb", [S, D], BF16, isOutput=False)
    qres = nc.declare_dram_parameter("qres", [SQ, D], F32, isOutput=False)
    wqT = nc.declare_dram_parameter("wqT", [D, D], BF16, isOutput=False)
    wkT = nc.declare_dram_parameter("wkT", [D, D], BF16, isOutput=False)
    wvT = nc.declare_dram_parameter("wvT", [D, D], BF16, isOutput=False)
    woT = nc.declare_dram_parameter("woT", [D, D], BF16, isOutput=False)
    bq_d = nc.declare_dram_parameter("bq", [D], F32, isOutput=False)
    bk_d = nc.declare_dram_parameter("bk", [D], F32, isOutput=False)
    bv_d = nc.declare_dram_parameter("bv", [D], F32, isOutput=False)
    bo_d = nc.declare_dram_parameter("bo", [D], F32, isOutput=False)
    gamma_d = nc.declare_dram_parameter("gamma", [D], F32, isOutput=False)
    beta_d = nc.declare_dram_parameter("beta", [D], F32, isOutput=False)

    attn_out = nc.declare_dram_parameter("attn_out", [H, SQ, S], F32, isOutput=True)
    out_rows = nc.declare_dram_parameter("out_rows", [SQ, D], F32, isOutput=True)

    with tile.TileContext(nc) as tc:
        with tc.tile_pool(name="res", bufs=1) as res, \
             tc.tile_pool(name="ps_mm", bufs=2, space="PSUM") as ps_mm, \
             tc.tile_pool(name="ps_ctx", bufs=2, space="PSUM") as ps_ctx, \
             tc.tile_pool(name="ps_tr", bufs=2, space="PSUM") as ps_tr:
            # ---------------- residents ----------------
            ident = res.tile([128, 128], BF16)
            make_identity(nc, ident)

            QT = res.tile([128, 8, SQ], BF16)      # [hd-part, i-chunk, q]
            KT = res.tile([128, 8, S], BF16)       # [hd-part, i-chunk, k]
            Vaug = res.tile([128, 16, H, HD + 1], BF16)  # [k-part, ktile, head, hd+1]
            ctxT = res.tile([128, 8, SQ], BF16)    # like QT
            woT_sb = res.tile([128, 8, D], BF16)   # [i-part, i-chunk, dout]

            bq_sb = res.tile([128, 8], F32)
            bk_sb = res.tile([128, 8], F32)
            bv_b = res.tile([128, D], F32)
            bo_b = res.tile([128, D], F32)
            g_b = res.tile([128, D], F32)
            be_b = res.tile([128, D], F32)
            eps = res.tile([128, 1], F32)

            nc.vector.memset(eps, 1e-5)
            # ones column for the denominators: fill all of Vaug with 1.0,
            # the projection copies below overwrite cols 0..63 per head.
            nc.vector.memset(Vaug, 1.0)

            for dst, src in ((bq_sb, bq_d), (bk_sb, bk_d)):
                nc.sync.dma_start(
                    out=dst,
                    in_=bass.AP(tensor=src[:].tensor, offset=0, ap=[[1, 128], [128, 8]]),
                )
            for dst, src in ((bv_b, bv_d), (bo_b, bo_d), (g_b, gamma_d), (be_b, beta_d)):
                nc.sync.dma_start(out=dst, in_=bcast_ap(src, 128, D))

            nc.sync.dma_start(
                out=woT_sb, in_=woT[:].rearrange("(c p) o -> p c o", p=128)
            )

            # ---------------- P0: transpose-loads + projections ----------------
            with tc.tile_pool(name="p0", bufs=1) as p0, \
                 tc.tile_pool(name="p0c", bufs=2) as p0c, \
                 tc.tile_pool(name="p0w", bufs=1) as p0w:
                # Q and K projections -> transposed layouts
                for xpb, s_x, wT_d, dest, bias_sb in (
                    (qpb, SQ, wqT, QT, bq_sb),
                    (kpb, S, wkT, KT, bk_sb),
                ):
                    xT = p0.tile([128, 8, S], BF16, tag="xT")
                    for dc in range(8):
                        nc.sync.dma_start(
                            out=xT[:, dc, 0:s_x],
                            in_=xpb[:, dc * 128:(dc + 1) * 128],
                            transpose=True,
                        )
                    for ic in range(8):
                        w_col = p0c.tile([128, 8, 128], BF16, tag="wcol")
                        nc.sync.dma_start(
                            out=w_col,
                            in_=wT_d[:, ic * 128:(ic + 1) * 128].rearrange(
                                "(c p) i -> p c i", p=128
                            ),
                        )
                        for sc in range(s_x // 512):
                            ps = ps_mm.tile([128, 512], F32, tag="mm")
                            for dc in range(8):
                                nc.tensor.matmul(
                                    ps,
                                    w_col[:, dc, :],
                                    xT[:, dc, sc * 512:(sc + 1) * 512],
                                    start=(dc == 0),
                                    stop=(dc == 7),
                                )
                            nc.scalar.activation(
                                out=dest[:, ic, sc * 512:(sc + 1) * 512],
                                in_=ps,
                                func=AF.Identity,
                                bias=bias_sb[:, ic:ic + 1],
                            )

                # V projection -> natural layout with ones column
                xT = p0.tile([128, 8, S], BF16, tag="xT")
                for dc in range(8):
                    nc.sync.dma_start(
                        out=xT[:, dc, :],
                        in_=vpb[:, dc * 128:(dc + 1) * 128],
                        transpose=True,
                    )
                wv_sb = p0w.tile([128, 8, D], BF16)
                nc.sync.dma_start(
                    out=wv_sb, in_=wvT[:].rearrange("(c p) i -> p c i", p=128)
                )
                for st in range(16):
                    for ic2 in range(2):
                        ps = ps_mm.tile([128, 512], F32, tag="mm")
                        for dc in range(8):
                            nc.tensor.matmul(
                                ps,
                                xT[:, dc, st * 128:(st + 1) * 128],
                                wv_sb[:, dc, ic2 * 512:(ic2 + 1) * 512],
                                start=(dc == 0),
                                stop=(dc == 7),
                            )
                        nc.vector.tensor_add(
                            out=Vaug[:, st, ic2 * 8:(ic2 + 1) * 8, 0:HD],
                            in0=ps.rearrange("p (h e) -> p h e", h=8),
                            in1=bv_b[:, ic2 * 512:(ic2 + 1) * 512].rearrange(
                                "p (h e) -> p h e", h=8
                            ),
                        )

            # ---------------- P1: attention, per (head, q-half) ----------------
            # Software-pipelined with a 1-gen lag: emit gen g's scores+exp,
            # then gen g-1's AV/denominators/transposes/stores - so the PE
            # always has the previous gen's work while ACT runs exp, keeping
            # the PE dense (HAM warm).
            with tc.tile_pool(name="p1e", bufs=2) as p1e, \
                 tc.tile_pool(name="p1s", bufs=3) as p1s, \
                 tc.tile_pool(name="p1m", bufs=3) as p1m, \
                 tc.tile_pool(name="p1d", bufs=3, space="DRAM") as p1d:

                def emit_head(h, qh):
                    hp = (h % 2) * 64
                    hc = h // 2
                    qoff = qh * 512
                    eT = p1e.tile([128, 16, 512], BF16, tag="expT")
                    for ktp in range(8):
                        ps = ps_mm.tile([128, 1024], F32, tag="mm")
                        for k2 in range(2):
                            kt = ktp * 2 + k2
                            nc.tensor.matmul(
                                ps[:, k2 * 512:(k2 + 1) * 512],
                                KT[hp:hp + 64, hc, kt * 128:(kt + 1) * 128],
                                QT[hp:hp + 64, hc, qoff:qoff + 512],
                                start=True,
                                stop=True,
                            )
                        nc.scalar.activation(
                            out=eT[:, ktp * 2:ktp * 2 + 2, :].rearrange(
                                "p a b -> p (a b)"
                            ),
                            in_=ps,
                            func=AF.Exp,
                            scale=0.125,
                        )
                    return eT

                def emit_tail(h, qh, eT):
                    hp = (h % 2) * 64
                    hc = h // 2
                    qoff = qh * 512
                    pc = ps_ctx.tile([HD + 1, 512], F32, tag="ctx")
                    for kt in range(16):
                        nc.tensor.matmul(
                            pc,
                            Vaug[:, kt, h, :],
                            eT[:, kt, :],
                            start=(kt == 0),
                            stop=(kt == 15),
                        )
                    # denominators: sums row -> DRAM -> {[128,4], [64,512]}
                    srow = p1m.tile([1, 512], F32, tag="row")
                    nc.scalar.copy(out=srow, in_=pc[HD:HD + 1, :])
                    scr = p1d.tile([1, 512], F32, tag="scr")
                    nc.sync.dma_start(out=scr, in_=srow)
                    sT = p1m.tile([128, 4], F32, tag="sT")
                    nc.sync.dma_start(
                        out=sT,
                        in_=bass.AP(
                            tensor=scr.tensor, offset=scr.offset,
                            ap=[[1, 128], [128, 4]],
                        ),
                    )
                    rsT = p1m.tile([128, 4], F32, tag="rsT")
                    nc.vector.reciprocal(out=rsT, in_=sT)
                    sb_b = p1m.tile([64, 512], F32, tag="sb_b")
                    nc.sync.dma_start(
                        out=sb_b,
                        in_=bass.AP(
                            tensor=scr.tensor, offset=scr.offset,
                            ap=[[0, 64], [1, 512]],
                        ),
                    )
                    rb = p1m.tile([64, 512], F32, tag="rb")
                    nc.vector.reciprocal(out=rb, in_=sb_b)
                    # normalized context rows for this head
                    nc.vector.tensor_mul(
                        out=ctxT[hp:hp + 64, hc, qoff:qoff + 512],
                        in0=pc[0:HD, :],
                        in1=rb,
                    )
                    # attn tiles: transpose exp back to [q, k], scale, store
                    for qt in range(4):
                        stg = p1s.tile([128, S], F32, tag="stage")
                        for grp in range(2):
                            ptr = ps_tr.tile([128, 1024], BF16, tag="tr")
                            for j in range(8):
                                kt = grp * 8 + j
                                nc.tensor.transpose(
                                    ptr[:, j * 128:(j + 1) * 128],
                                    eT[:, kt, qt * 128:(qt + 1) * 128],
                                    ident,
                                )
                            nc.vector.tensor_scalar_mul(
                                out=stg[:, grp * 1024:(grp + 1) * 1024],
                                in0=ptr,
                                scalar1=rsT[:, qt:qt + 1],
                            )
                        nc.sync.dma_start(
                            out=attn_out[h, qoff + qt * 128:qoff + (qt + 1) * 128, :],
                            in_=stg,
                        )

                prev = None
                for h in range(H):
                    for qh in range(2):
                        eT = emit_head(h, qh)
                        if prev is not None:
                            emit_tail(*prev)
                        prev = (h, qh, eT)
                emit_tail(*prev)

            # ---------------- P2: out-projection + residual + LayerNorm --------
            with tc.tile_pool(name="p2", bufs=2) as p2:
                for st in range(8):
                    po = ps_mm.tile([128, 1024], F32, tag="mm")
                    for oc in range(2):
                        for ic in range(8):
                            nc.tensor.matmul(
                                po[:, oc * 512:(oc + 1) * 512],
                                ctxT[:, ic, st * 128:(st + 1) * 128],
                                woT_sb[:, ic, oc * 512:(oc + 1) * 512],
                                start=(ic == 0),
                                stop=(ic == 7),
                            )
                    qrt = p2.tile([128, D], F32, tag="qres")
                    nc.sync.dma_start(
                        out=qrt, in_=qres[st * 128:(st + 1) * 128, :]
                    )
                    xt2 = p2.tile([128, D], F32, tag="xt")
                    nc.vector.tensor_add(out=xt2, in0=po, in1=bo_b)
                    nc.vector.tensor_add(out=xt2, in0=xt2, in1=qrt)
                    xv = xt2.rearrange("p (s f) -> p s f", s=2)
                    stats = p2.tile([128, 2, 6], F32, tag="stats")
                    for s_ in range(2):
                        nc.vector.bn_stats(out=stats[:, s_, :], in_=xv[:, s_, :])
                    mv = p2.tile([128, 2], F32, tag="mv")
                    nc.vector.bn_aggr(out=mv, in_=stats)
                    std = p2.tile([128, 1], F32, tag="std")
                    nc.scalar.activation(
                        out=std, in_=mv[:, 1:2], func=AF.Sqrt, bias=eps
                    )
                    rstd = p2.tile([128, 1], F32, tag="rstd")
                    nc.vector.reciprocal(out=rstd, in_=std)
                    nc.vector.tensor_scalar(
                        out=xt2, in0=xt2, scalar1=mv[:, 0:1], scalar2=rstd,
                        op0=ALU.subtract, op1=ALU.mult,
                    )
                    nc.vector.tensor_mul(out=xt2, in0=xt2, in1=g_b)
                    nc.vector.tensor_add(out=xt2, in0=xt2, in1=be_b)
                    nc.sync.dma_start(
                        out=out_rows[st * 128:(st + 1) * 128, :], in_=xt2
                    )

    split_excess_waits(nc)
    return nc


def split_excess_waits(nc, cap=1):
    """This walrus build rejects >cap sem waits on one instruction ("Too many
    sync wait commands"). Move excess waits onto preceding NoOps on the same
    engine - engine program order makes this equivalent."""
    n_split = 0
    for fn in nc.m.functions:
        for blk in fn.blocks:
            if not any(
                inst.sync_info is not None and len(inst.sync_info.on_wait) > cap
                for inst in blk.instructions
            ):
                continue
            newins = []
            for inst in blk.instructions:
                si = inst.sync_info
                waits = list(si.on_wait) if si is not None else []
                if len(waits) > cap:
                    extra, keep = waits[:-cap], waits[-cap:]
                    for w in extra:
                        nop = mybir.InstNoOp(
                            name=f"{inst.name}-ws{n_split}", ins=[], outs=[]
                        )
                        nop.engine = inst.engine
                        nop.sync_info = mybir.SyncInfo(on_wait=[w], on_update=[])
                        newins.append(nop)
                        n_split += 1
                    inst.sync_info = mybir.SyncInfo(
                        on_wait=keep, on_update=list(si.on_update)
                    )
                newins.append(inst)
            blk.instructions = newins
    return n_split


_NC_CACHE = None


def _get_nc():
    global _NC_CACHE
    if _NC_CACHE is None:
        _NC_CACHE = build_nc()
    return _NC_CACHE


def make_in_maps(query, key_t, value, Wq, bq, Wk, bk, Wv, bv, Wo, bo, gamma, beta):
    bf16 = ml_dtypes.bfloat16
    query = np.asarray(query, dtype=np.float32)
    key_t = np.asarray(key_t, dtype=np.float32)
    value = np.asarray(value, dtype=np.float32)
    shared = {
        "wqT": np.ascontiguousarray(np.asarray(Wq, np.float32).T).astype(bf16),
        "wkT": np.ascontiguousarray(np.asarray(Wk, np.float32).T).astype(bf16),
        "wvT": np.ascontiguousarray(np.asarray(Wv, np.float32).T).astype(bf16),
        "woT": np.ascontiguousarray(np.asarray(Wo, np.float32).T).astype(bf16),
        "bq": np.asarray(bq, np.float32),
        "bk": np.asarray(bk, np.float32),
        "bv": np.asarray(bv, np.float32),
        "bo": np.asarray(bo, np.float32),
        "gamma": np.asarray(gamma, np.float32),
        "beta": np.asarray(beta, np.float32),
    }
    in_maps = []
    for c in range(N_CORES):
        b, hf = c // 2, c % 2
        rows = slice(hf * SQ, (hf + 1) * SQ)
        in_maps.append(
            {
                "qpb": query[b, rows].astype(bf16),
                "kpb": key_t[b].astype(bf16),
                "vpb": value[b].astype(bf16),
                "qres": np.ascontiguousarray(query[b, rows]),
                **shared,
            }
        )
    return in_maps


def run(in_maps, **kw):
    nc = _get_nc()
    return run_bass_kernel_spmd(nc, in_maps, list(range(N_CORES)), **kw)


def assemble(results):
    out = np.empty((B, S, D), np.float32)
    attn = np.empty((B, H, S, S), np.float32)
    for c in range(N_CORES):
        b, hf = c // 2, c % 2
        rows = slice(hf * SQ, (hf + 1) * SQ)
        out[b, rows, :] = results[c]["out_rows"]
        attn[b, :, rows, :] = results[c]["attn_out"]
    return out, attn


def kernel(query, key_t, value, Wq, bq, Wk, bk, Wv, bv, Wo, bo, gamma, beta):
    in_maps = make_in_maps(
        query, key_t, value, Wq, bq, Wk, bk, Wv, bv, Wo, bo, gamma, beta
    )
    res = run(in_maps)
    return assemble(res.results)


# revision 5
# speedup vs baseline: 1.1218x; 1.0746x over previous
"""Trainium2 Bass kernel for nn_MultiHeadAttention (B=4, S=2048, D=1024, H=16).

Sharding: 8 cores = batch (4) x query-row-halves (2). Each core computes, for
its batch b and its 1024 query rows: the QKV projections (K/V over the full
2048 key rows), all 16 heads of attention, the output projection, residual add
and LayerNorm. No collectives needed - every core's output rows depend only on
its own data. The full attention-probability tensor [4,16,2048,2048] is a
kernel output; each core writes its [16,1024,2048] slice.

Per-core dataflow (all matmul inputs bf16, fp32 PSUM accumulation):
  P0: DMA-transpose-load inputs (d on partitions), project to
      QT/KT [head_dim-major, seq] and V [seq-major, head_dim] with a ones
      column appended per head (yields softmax denominators for free).
  P1: per (head, q-half) generation: scores^T = K Q^T / 8 on PE, exp on ACT,
      attn@V accumulation on PE (ones row -> denominators), PE-transpose of
      the exp tiles back to [q, k] orientation, normalize on DVE, DMA out.
      Context rows are normalized per head (pre head-mix) via a DRAM-scratch
      broadcast of 1/sum.
  P2: out-projection on PE, + bias + residual, LayerNorm (bn_stats/bn_aggr),
      DMA out rows.
"""

import sys

sys.path.insert(0, "/opt/trn_rl_repo")

import numpy as np
import ml_dtypes

import concourse.bass as bass
import concourse.mybir as mybir
import concourse.tile as tile
from concourse.masks import make_identity
from concourse.bass_utils import run_bass_kernel_spmd

F32 = mybir.dt.float32
BF16 = mybir.dt.bfloat16
AF = mybir.ActivationFunctionType
ALU = mybir.AluOpType

B, S, D, H = 4, 2048, 1024, 16
HD = D // H          # 64
SQ = S // 2          # local query rows per core = 1024
N_CORES = 8


def bcast_ap(dram_param, parts, free):
    """DRAM [free] -> AP replicated across `parts` partitions."""
    return bass.AP(tensor=dram_param[:].tensor, offset=0, ap=[[0, parts], [1, free]])


def build_nc():
    nc = bass.Bass()

    qpb = nc.declare_dram_parameter("qpb", [SQ, D], BF16, isOutput=False)
    kpb = nc.declare_dram_parameter("kpb", [S, D], BF16, isOutput=False)
    vpb = nc.declare_dram_parameter("vpb", [S, D], BF16, isOutput=False)
    qres = nc.declare_dram_parameter("qres", [SQ, D], F32, isOutput=False)
    wqT = nc.declare_dram_parameter("wqT", [D, D], BF16, isOutput=False)
    wkT = nc.declare_dram_parameter("wkT", [D, D], BF16, isOutput=False)
    wvT = nc.declare_dram_parameter("wvT", [D, D], BF16, isOutput=False)
    woT = nc.declare_dram_parameter("woT", [D, D], BF16, isOutput=False)
    bq_d = nc.declare_dram_parameter("bq", [D], F32, isOutput=False)
    bk_d = nc.declare_dram_parameter("bk", [D], F32, isOutput=False)
    bv_d = nc.declare_dram_parameter("bv", [D], F32, isOutput=False)
    bo_d = nc.declare_dram_parameter("bo", [D], F32, isOutput=False)
    gamma_d = nc.declare_dram_parameter("gamma", [D], F32, isOutput=False)
    beta_d = nc.declare_dram_parameter("beta", [D], F32, isOutput=False)

    attn_out = nc.declare_dram_parameter("attn_out", [H, SQ, S], F32, isOutput=True)
    out_rows = nc.declare_dram_parameter("out_rows", [SQ, D], F32, isOutput=True)

    with tile.TileContext(nc) as tc:
        with tc.tile_pool(name="res", bufs=1) as res, \
             tc.tile_pool(name="ps_mm", bufs=2, space="PSUM") as ps_mm, \
             tc.tile_pool(name="ps_ctx", bufs=1, space="PSUM") as ps_ctx, \
             tc.tile_pool(name="ps_tr", bufs=3, space="PSUM") as ps_tr:
            # ---------------- residents ----------------
            ident = res.tile([128, 128], BF16)
            make_identity(nc, ident)

            QT = res.tile([128, 8, SQ], BF16)      # [hd-part, i-chunk, q]
            KT = res.tile([128, 8, S], BF16)       # [hd-part, i-chunk, k]
            Vaug = res.tile([128, 16, H, HD + 1], BF16)  # [k-part, ktile, head, hd+1]
            ctxT = res.tile([128, 8, SQ], BF16)    # like QT
            woT_sb = res.tile([128, 8, D], BF16)   # [i-part, i-chunk, dout]

            bq_sb = res.tile([128, 8], F32)
            bk_sb = res.tile([128, 8], F32)
            bv_b = res.tile([128, D], F32)
            bo_b = res.tile([128, D], F32)
            g_b = res.tile([128, D], F32)
            be_b = res.tile([128, D], F32)
            eps = res.tile([128, 1], F32)

            nc.vector.memset(eps, 1e-5)
            # ones column for the denominators: fill all of Vaug with 1.0,
            # the projection copies below overwrite cols 0..63 per head.
            nc.vector.memset(Vaug, 1.0)

            for dst, src in ((bq_sb, bq_d), (bk_sb, bk_d)):
                nc.sync.dma_start(
                    out=dst,
                    in_=bass.AP(tensor=src[:].tensor, offset=0, ap=[[1, 128], [128, 8]]),
                )
            for dst, src in ((bv_b, bv_d), (bo_b, bo_d), (g_b, gamma_d), (be_b, beta_d)):
                nc.sync.dma_start(out=dst, in_=bcast_ap(src, 128, D))

            nc.sync.dma_start(
                out=woT_sb, in_=woT[:].rearrange("(c p) o -> p c o", p=128)
            )

            # ---------------- P0: transpose-loads + projections ----------------
            with tc.tile_pool(name="p0", bufs=1) as p0, \
                 tc.tile_pool(name="p0c", bufs=2) as p0c, \
                 tc.tile_pool(name="p0w", bufs=1) as p0w:
                # Q and K projections -> transposed layouts
                for xpb, s_x, wT_d, dest, bias_sb in (
                    (qpb, SQ, wqT, QT, bq_sb),
                    (kpb, S, wkT, KT, bk_sb),
                ):
                    xT = p0.tile([128, 8, S], BF16, tag="xT")
                    for dc in range(8):
                        nc.sync.dma_start(
                            out=xT[:, dc, 0:s_x],
                            in_=xpb[:, dc * 128:(dc + 1) * 128],
                            transpose=True,
                        )
                    for ic in range(8):
                        w_col = p0c.tile([128, 8, 128], BF16, tag="wcol")
                        nc.sync.dma_start(
                            out=w_col,
                            in_=wT_d[:, ic * 128:(ic + 1) * 128].rearrange(
                                "(c p) i -> p c i", p=128
                            ),
                        )
                        for sc in range(s_x // 512):
                            ps = ps_mm.tile([128, 512], F32, tag="mm")
                            for dc in range(8):
                                nc.tensor.matmul(
                                    ps,
                                    w_col[:, dc, :],
                                    xT[:, dc, sc * 512:(sc + 1) * 512],
                                    start=(dc == 0),
                                    stop=(dc == 7),
                                )
                            nc.scalar.activation(
                                out=dest[:, ic, sc * 512:(sc + 1) * 512],
                                in_=ps,
                                func=AF.Identity,
                                bias=bias_sb[:, ic:ic + 1],
                            )

                # V projection -> natural layout with ones column
                xT = p0.tile([128, 8, S], BF16, tag="xT")
                for dc in range(8):
                    nc.sync.dma_start(
                        out=xT[:, dc, :],
                        in_=vpb[:, dc * 128:(dc + 1) * 128],
                        transpose=True,
                    )
                wv_sb = p0w.tile([128, 8, D], BF16)
                nc.sync.dma_start(
                    out=wv_sb, in_=wvT[:].rearrange("(c p) i -> p c i", p=128)
                )
                for st in range(16):
                    for ic2 in range(2):
                        ps = ps_mm.tile([128, 512], F32, tag="mm")
                        for dc in range(8):
                            nc.tensor.matmul(
                                ps,
                                xT[:, dc, st * 128:(st + 1) * 128],
                                wv_sb[:, dc, ic2 * 512:(ic2 + 1) * 512],
                                start=(dc == 0),
                                stop=(dc == 7),
                            )
                        nc.vector.tensor_add(
                            out=Vaug[:, st, ic2 * 8:(ic2 + 1) * 8, 0:HD],
                            in0=ps.rearrange("p (h e) -> p h e", h=8),
                            in1=bv_b[:, ic2 * 512:(ic2 + 1) * 512].rearrange(
                                "p (h e) -> p h e", h=8
                            ),
                        )

            # ---------------- P1: attention, per (head, q-half) ----------------
            # Software-pipelined with a 1-gen lag: emit gen g's scores+exp,
            # then gen g-1's AV/denominators/transposes/stores - so the PE
            # always has the previous gen's work while ACT runs exp, keeping
            # the PE dense (HAM warm).
            with tc.tile_pool(name="p1e", bufs=2) as p1e, \
                 tc.tile_pool(name="p1s", bufs=3) as p1s, \
                 tc.tile_pool(name="p1m", bufs=3) as p1m, \
                 tc.tile_pool(name="p1d", bufs=3, space="DRAM") as p1d:

                def emit_head(h, qh):
                    hp = (h % 2) * 64
                    hc = h // 2
                    qoff = qh * 512
                    eT = p1e.tile([128, 16, 512], BF16, tag="expT")
                    for ktp in range(8):
                        ps = ps_mm.tile([128, 1024], F32, tag="mm")
                        for k2 in range(2):
                            kt = ktp * 2 + k2
                            for mh in range(2):
                                nc.tensor.matmul(
                                    ps[mh * 64:(mh + 1) * 64, k2 * 512:(k2 + 1) * 512],
                                    KT[hp:hp + 64, hc,
                                       kt * 128 + mh * 64:kt * 128 + (mh + 1) * 64],
                                    QT[hp:hp + 64, hc, qoff:qoff + 512],
                                    start=True,
                                    stop=True,
                                )
                        nc.scalar.activation(
                            out=eT[:, ktp * 2:ktp * 2 + 2, :].rearrange(
                                "p a b -> p (a b)"
                            ),
                            in_=ps,
                            func=AF.Exp,
                            scale=0.125,
                        )
                    return eT

                def emit_tail(h, qh, eT):
                    hp = (h % 2) * 64
                    hc = h // 2
                    qoff = qh * 512
                    pc = ps_ctx.tile([HD + 1, 512], F32, tag="ctx")
                    for kt in range(16):
                        nc.tensor.matmul(
                            pc,
                            Vaug[:, kt, h, :],
                            eT[:, kt, :],
                            start=(kt == 0),
                            stop=(kt == 15),
                        )
                    # denominators: sums row -> DRAM -> {[128,4], [64,512]}
                    srow = p1m.tile([1, 512], F32, tag="row")
                    nc.scalar.copy(out=srow, in_=pc[HD:HD + 1, :])
                    scr = p1d.tile([1, 512], F32, tag="scr")
                    nc.sync.dma_start(out=scr, in_=srow)
                    sT = p1m.tile([128, 4], F32, tag="sT")
                    nc.sync.dma_start(
                        out=sT,
                        in_=bass.AP(
                            tensor=scr.tensor, offset=scr.offset,
                            ap=[[1, 128], [128, 4]],
                        ),
                    )
                    rsT = p1m.tile([128, 4], F32, tag="rsT")
                    nc.vector.reciprocal(out=rsT, in_=sT)
                    sb_b = p1m.tile([64, 512], F32, tag="sb_b")
                    nc.sync.dma_start(
                        out=sb_b,
                        in_=bass.AP(
                            tensor=scr.tensor, offset=scr.offset,
                            ap=[[0, 64], [1, 512]],
                        ),
                    )
                    rb = p1m.tile([64, 512], F32, tag="rb")
                    nc.vector.reciprocal(out=rb, in_=sb_b)
                    # normalized context rows for this head
                    nc.vector.tensor_mul(
                        out=ctxT[hp:hp + 64, hc, qoff:qoff + 512],
                        in0=pc[0:HD, :],
                        in1=rb,
                    )
                    # attn tiles: transpose exp back to [q, k], scale, store
                    for qt in range(4):
                        stg = p1s.tile([128, S], F32, tag="stage")
                        for grp in range(2):
                            ptr = ps_tr.tile([128, 1024], BF16, tag="tr")
                            for j in range(8):
                                kt = grp * 8 + j
                                nc.tensor.transpose(
                                    ptr[:, j * 128:(j + 1) * 128],
                                    eT[:, kt, qt * 128:(qt + 1) * 128],
                                    ident,
                                )
                            if grp == 0:
                                nc.vector.tensor_scalar_mul(
                                    out=stg[:, grp * 1024:(grp + 1) * 1024],
                                    in0=ptr,
                                    scalar1=rsT[:, qt:qt + 1],
                                )
                            else:
                                nc.scalar.activation(
                                    out=stg[:, grp * 1024:(grp + 1) * 1024],
                                    in_=ptr,
                                    func=AF.Copy,
                                    scale=rsT[:, qt:qt + 1],
                                )
                        nc.sync.dma_start(
                            out=attn_out[h, qoff + qt * 128:qoff + (qt + 1) * 128, :],
                            in_=stg,
                        )

                prev = None
                for h in range(H):
                    for qh in range(2):
                        eT = emit_head(h, qh)
                        if prev is not None:
                            emit_tail(*prev)
                        prev = (h, qh, eT)
                emit_tail(*prev)

            # ---------------- P2: out-projection + residual + LayerNorm --------
            with tc.tile_pool(name="p2", bufs=2) as p2:
                for st in range(8):
                    po = ps_mm.tile([128, 1024], F32, tag="mm")
                    for oc in range(2):
                        for ic in range(8):
                            nc.tensor.matmul(
                                po[:, oc * 512:(oc + 1) * 512],
                                ctxT[:, ic, st * 128:(st + 1) * 128],
                                woT_sb[:, ic, oc * 512:(oc + 1) * 512],
                                start=(ic == 0),
                                stop=(ic == 7),
                            )
                    qrt = p2.tile([128, D], F32, tag="qres")
                    nc.sync.dma_start(
                        out=qrt, in_=qres[st * 128:(st + 1) * 128, :]
                    )
                    xt2 = p2.tile([128, D], F32, tag="xt")
                    nc.vector.tensor_add(out=xt2, in0=po, in1=bo_b)
                    nc.vector.tensor_add(out=xt2, in0=xt2, in1=qrt)
                    xv = xt2.rearrange("p (s f) -> p s f", s=2)
                    stats = p2.tile([128, 2, 6], F32, tag="stats")
                    for s_ in range(2):
                        nc.vector.bn_stats(out=stats[:, s_, :], in_=xv[:, s_, :])
                    mv = p2.tile([128, 2], F32, tag="mv")
                    nc.vector.bn_aggr(out=mv, in_=stats)
                    std = p2.tile([128, 1], F32, tag="std")
                    nc.scalar.activation(
                        out=std, in_=mv[:, 1:2], func=AF.Sqrt, bias=eps
                    )
                    rstd = p2.tile([128, 1], F32, tag="rstd")
                    nc.vector.reciprocal(out=rstd, in_=std)
                    nc.vector.tensor_scalar(
                        out=xt2, in0=xt2, scalar1=mv[:, 0:1], scalar2=rstd,
                        op0=ALU.subtract, op1=ALU.mult,
                    )
                    nc.vector.tensor_mul(out=xt2, in0=xt2, in1=g_b)
                    nc.vector.tensor_add(out=xt2, in0=xt2, in1=be_b)
                    nc.sync.dma_start(
                        out=out_rows[st * 128:(st + 1) * 128, :], in_=xt2
                    )

    split_excess_waits(nc)
    return nc


def split_excess_waits(nc, cap=1):
    """This walrus build rejects >cap sem waits on one instruction ("Too many
    sync wait commands"). Move excess waits onto preceding NoOps on the same
    engine - engine program order makes this equivalent."""
    n_split = 0
    for fn in nc.m.functions:
        for blk in fn.blocks:
            if not any(
                inst.sync_info is not None and len(inst.sync_info.on_wait) > cap
                for inst in blk.instructions
            ):
                continue
            newins = []
            for inst in blk.instructions:
                si = inst.sync_info
                waits = list(si.on_wait) if si is not None else []
                if len(waits) > cap:
                    extra, keep = waits[:-cap], waits[-cap:]
                    for w in extra:
                        nop = mybir.InstNoOp(
                            name=f"{inst.name}-ws{n_split}", ins=[], outs=[]
                        )
                        nop.engine = inst.engine
                        nop.sync_info = mybir.SyncInfo(on_wait=[w], on_update=[])
                        newins.append(nop)
                        n_split += 1
                    inst.sync_info = mybir.SyncInfo(
                        on_wait=keep, on_update=list(si.on_update)
                    )
                newins.append(inst)
            blk.instructions = newins
    return n_split


_NC_CACHE = None


def _get_nc():
    global _NC_CACHE
    if _NC_CACHE is None:
        _NC_CACHE = build_nc()
    return _NC_CACHE


def make_in_maps(query, key_t, value, Wq, bq, Wk, bk, Wv, bv, Wo, bo, gamma, beta):
    bf16 = ml_dtypes.bfloat16
    query = np.asarray(query, dtype=np.float32)
    key_t = np.asarray(key_t, dtype=np.float32)
    value = np.asarray(value, dtype=np.float32)
    shared = {
        "wqT": np.ascontiguousarray(np.asarray(Wq, np.float32).T).astype(bf16),
        "wkT": np.ascontiguousarray(np.asarray(Wk, np.float32).T).astype(bf16),
        "wvT": np.ascontiguousarray(np.asarray(Wv, np.float32).T).astype(bf16),
        "woT": np.ascontiguousarray(np.asarray(Wo, np.float32).T).astype(bf16),
        "bq": np.asarray(bq, np.float32),
        "bk": np.asarray(bk, np.float32),
        "bv": np.asarray(bv, np.float32),
        "bo": np.asarray(bo, np.float32),
        "gamma": np.asarray(gamma, np.float32),
        "beta": np.asarray(beta, np.float32),
    }
    in_maps = []
    for c in range(N_CORES):
        b, hf = c // 2, c % 2
        rows = slice(hf * SQ, (hf + 1) * SQ)
        in_maps.append(
            {
                "qpb": query[b, rows].astype(bf16),
                "kpb": key_t[b].astype(bf16),
                "vpb": value[b].astype(bf16),
                "qres": np.ascontiguousarray(query[b, rows]),
                **shared,
            }
        )
    return in_maps


def run(in_maps, **kw):
    nc = _get_nc()
    return run_bass_kernel_spmd(nc, in_maps, list(range(N_CORES)), **kw)


def assemble(results):
    out = np.empty((B, S, D), np.float32)
    attn = np.empty((B, H, S, S), np.float32)
    for c in range(N_CORES):
        b, hf = c // 2, c % 2
        rows = slice(hf * SQ, (hf + 1) * SQ)
        out[b, rows, :] = results[c]["out_rows"]
        attn[b, :, rows, :] = results[c]["attn_out"]
    return out, attn


def kernel(query, key_t, value, Wq, bq, Wk, bk, Wv, bv, Wo, bo, gamma, beta):
    in_maps = make_in_maps(
        query, key_t, value, Wq, bq, Wk, bk, Wv, bv, Wo, bo, gamma, beta
    )
    res = run(in_maps)
    return assemble(res.results)
